# revision 5
# baseline (speedup 1.0000x reference)
"""CapsNet dynamic-routing layer on 8 Trainium2 NeuronCores.

Math (per example, S=512 input capsules of dim D=256, 16 output capsules of
dim 32, O = 16*32 = 512):
  u_hat = x @ W                     # [S, O]
  b = 0; for 3 routing iters:
    c = softmax_n(b)                # over the 16-capsule axis
    s[n] = sum_s c[n,s] * u_hat[s, n*32:(n+1)*32]
    v = s / sqrt(|s|^2 + 1e-7)
    b[n,s] = v[n] . u_hat[s, n*32:(n+1)*32]
  out = v.flatten()

Sharding: pure data-parallel over the batch (256 examples -> 32 per core),
W replicated, no cross-core communication.

Layouts on chip (per example):
  u_hat  [S, O]  (S on partitions, 4 tiles) - rhs of the s-matmul
  u_hatT [O, S]  (O on partitions, 4 tiles) - rhs of the b-update matmul
Both are produced directly by the tensor engine from xT = x.T (host-side
transpose) since both matmuls contract over D:
  u_hat  tile m: sum_k  xT[k-tile][:, m-slice].T @ W[k-tile]
  u_hatT tile m: sum_k  W[k-tile][:, m-slice].T  @ xT[k-tile]
b is kept transposed as bT [S, 16] so softmax runs along the free dim.
"""

import sys

sys.path.insert(0, "/opt/trn_rl_repo")

import numpy as np

import concourse.bacc as bacc
import concourse.mybir as mybir
import concourse.tile as tile
from concourse import bass
from concourse.bass_utils import run_bass_kernel_spmd
from concourse.masks import make_identity

F32 = mybir.dt.float32
AX = mybir.AxisListType
AF = mybir.ActivationFunctionType

B, S, D = 256, 512, 256
NC_, DC = 16, 32  # num_capsule, dim_capsule
O = NC_ * DC  # 512
N_CORES = 8
E = B // N_CORES  # 32 examples per core
ROUTINGS = 3
KT_D = D // 128  # 2 k-tiles over D
MT = 4  # 4 tiles over S and over O


def host_masks():
    # dmask[n, n'*32+d] = (n' == n): diagonal-block extraction mask
    dmask = np.zeros((NC_, O), np.float32)
    for n in range(NC_):
        dmask[n, n * DC : (n + 1) * DC] = 1.0
    # vmask[q, k*16+n'] = (n' == 4k + q//32): builds block-diag V from vT-replica
    vmask = np.zeros((128, 4 * NC_), np.float32)
    for q in range(128):
        for k in range(4):
            vmask[q, k * NC_ + 4 * k + q // 32] = 1.0
    return dmask, vmask


def emit_example(nc, pools, consts, xT_ap, out_ap, e):
    (xp, up, utp, sp, ctp, pcre, prt, ptr, pvt) = pools
    (W_t, dmask_t, vmask_t, ident_t, c0_t, eps_t) = consts

    # ---- load xT for this example: [D, S] as 2 partition tiles ----
    xt = []
    for k in range(KT_D):
        t = xp.tile([128, S], F32, tag=f"xt{k}")
        nc.sync.dma_start(t[:], xT_ap[e, 128 * k : 128 * (k + 1), :])
        xt.append(t)

    # ---- u_hat [S, O] and u_hatT [O, S] ----
    uh, uhT = [], []
    for m in range(MT):
        pu = pcre.tile([128, O], F32, tag="pcre")
        for k in range(KT_D):
            nc.tensor.matmul(
                pu[:],
                xt[k][:, bass.ts(m, 128)],
                W_t[k][:],
                start=(k == 0),
                stop=(k == KT_D - 1),
            )
        t = up.tile([128, O], F32, tag=f"uh{m}")
        nc.scalar.copy(t[:], pu[:])
        uh.append(t)
    for m in range(MT):
        pu = pcre.tile([128, S], F32, tag="pcre")
        for k in range(KT_D):
            nc.tensor.matmul(
                pu[:],
                W_t[k][:, bass.ts(m, 128)],
                xt[k][:],
                start=(k == 0),
                stop=(k == KT_D - 1),
            )
        t = utp.tile([128, S], F32, tag=f"uht{m}")
        nc.scalar.copy(t[:], pu[:])
        uhT.append(t)

    # ---- routing ----
    cT = None  # [S, 16] as 4 tiles; iter 0 uses the constant 1/16 tile
    v = None
    for it in range(ROUTINGS):
        # s_full = cT.T @ u_hat : [16, O]
        ps = prt.tile([NC_, O], F32, tag="ps")
        for m in range(MT):
            lhs = c0_t if cT is None else cT[m]
            nc.tensor.matmul(
                ps[:], lhs[:], uh[m][:], start=(m == 0), stop=(m == MT - 1)
            )
        # diagonal-block extraction: s[n, d] = sum_n' masked[n, n'*32+d]
        masked = sp.tile([NC_, O], F32, tag="masked")
        nc.vector.tensor_mul(masked[:], ps[:], dmask_t[:])
        s = sp.tile([NC_, DC], F32, tag="s")
        nc.vector.tensor_reduce(
            s[:],
            masked[:].rearrange("p (n d) -> p d n", n=NC_),
            axis=AX.X,
            op=mybir.AluOpType.add,
        )
        # squash: v = s / sqrt(sum_d s^2 + 1e-7)
        sq = sp.tile([NC_, DC], F32, tag="sq")
        ss = sp.tile([NC_, 1], F32, tag="ss")
        nc.scalar.activation(sq[:], s[:], AF.Square, accum_out=ss[:])
        nrm = sp.tile([NC_, 1], F32, tag="nrm")
        nc.scalar.activation(nrm[:], ss[:], AF.Sqrt, bias=eps_t[:NC_, :])
        inv = sp.tile([NC_, 1], F32, tag="inv")
        nc.vector.reciprocal(inv[:], nrm[:])
        v = sp.tile([NC_, DC], F32, tag="v")
        nc.vector.tensor_scalar_mul(v[:], s[:], inv[:])

        if it == ROUTINGS - 1:
            break

        # ---- b update: bT' = (Vblk.T @ u_hatT) : [16, S] ----
        # Vblk [O, 16] block-diagonal, built from v via transpose + replicate + mask
        vtp = pvt.tile([DC, NC_], F32, tag="vtp")
        nc.tensor.transpose(vtp[:], v[:], ident_t[:NC_, :NC_])
        vrep = sp.tile([128, NC_], F32, tag="vrep")
        for r in range(4):
            nc.scalar.copy(vrep[32 * r : 32 * (r + 1), :], vtp[:])
        vblk = sp.tile([128, 4 * NC_], F32, tag="vblk")
        for k in range(4):
            nc.vector.tensor_mul(
                vblk[:, bass.ts(k, NC_)], vmask_t[:, bass.ts(k, NC_)], vrep[:]
            )
        pb = prt.tile([NC_, S], F32, tag="ps")
        for k in range(MT):
            nc.tensor.matmul(
                pb[:],
                vblk[:, bass.ts(k, NC_)],
                uhT[k][:],
                start=(k == 0),
                stop=(k == MT - 1),
            )
        # softmax over n (free dim after transpose):
        # exp on the [16, S] layout, transpose 128-col slices to [S-tile, 16]
        expb = sp.tile([NC_, S], F32, tag="expb")
        nc.scalar.activation(expb[:], pb[:], AF.Exp)
        r4 = sp.tile([128, MT], F32, tag="r4")
        et_sb = []
        for m in range(MT):
            et = ptr.tile([128, NC_], F32, tag="et")
            nc.tensor.transpose(et[:], expb[:, bass.ts(m, 128)], ident_t[:NC_, :NC_])
            nc.vector.reduce_sum(r4[:, m : m + 1], et[:], axis=AX.X)
            esb = sp.tile([128, NC_], F32, tag=f"esb{m}")
            nc.scalar.copy(esb[:], et[:])
            et_sb.append(esb)
        inv4 = sp.tile([128, MT], F32, tag="inv4")
        nc.vector.reciprocal(inv4[:], r4[:])
        cT = []
        for m in range(MT):
            ct = ctp.tile([128, NC_], F32, tag=f"ct{m}")
            nc.vector.tensor_scalar_mul(ct[:], et_sb[m][:], inv4[:, m : m + 1])
            cT.append(ct)

    # ---- output: v [16, 32] -> out[e] (row of 512) ----
    nc.sync.dma_start(out_ap[e].rearrange("(n d) -> n d", n=NC_), v[:])


def build(n_ex=E, num_devices=N_CORES):
    nc = bacc.Bacc(
        "TRN2", target_bir_lowering=False, debug=False, num_devices=num_devices
    )
    xT_d = nc.dram_tensor("xT", [n_ex, D, S], F32, kind="ExternalInput")
    W_d = nc.dram_tensor("W", [D, O], F32, kind="ExternalInput")
    dmask_d = nc.dram_tensor("dmask", [NC_, O], F32, kind="ExternalInput")
    vmask_d = nc.dram_tensor("vmask", [128, 4 * NC_], F32, kind="ExternalInput")
    out_d = nc.dram_tensor("out", [n_ex, O], F32, kind="ExternalOutput")

    with tile.TileContext(nc) as tc:
        with (
            tc.tile_pool(name="consts", bufs=1) as cp,
            tc.tile_pool(name="xp", bufs=2) as xp,
            tc.tile_pool(name="up", bufs=2) as up,
            tc.tile_pool(name="utp", bufs=2) as utp,
            tc.tile_pool(name="sp", bufs=3) as sp,
            tc.tile_pool(name="ctp", bufs=2) as ctp,
            tc.tile_pool(name="pcre", bufs=2, space=bass.MemorySpace.PSUM) as pcre,
            tc.tile_pool(name="prt", bufs=2, space=bass.MemorySpace.PSUM) as prt,
            tc.tile_pool(name="ptr", bufs=2, space=bass.MemorySpace.PSUM) as ptr,
            tc.tile_pool(name="pvt", bufs=1, space=bass.MemorySpace.PSUM) as pvt,
        ):
            W_t = []
            for k in range(KT_D):
                t = cp.tile([128, O], F32, tag=f"W{k}")
                nc.sync.dma_start(t[:], W_d.ap()[128 * k : 128 * (k + 1), :])
                W_t.append(t)
            dmask_t = cp.tile([NC_, O], F32, tag="dmask")
            nc.sync.dma_start(dmask_t[:], dmask_d.ap())
            vmask_t = cp.tile([128, 4 * NC_], F32, tag="vmask")
            nc.sync.dma_start(vmask_t[:], vmask_d.ap())
            ident_t = cp.tile([128, 128], F32, tag="ident")
            make_identity(nc, ident_t[:])
            c0_t = cp.tile([128, NC_], F32, tag="c0")
            nc.vector.memset(c0_t[:], 1.0 / NC_)
            eps_t = cp.tile([128, 1], F32, tag="eps")
            nc.vector.memset(eps_t[:], 1e-7)

            pools = (xp, up, utp, sp, ctp, pcre, prt, ptr, pvt)
            consts = (W_t, dmask_t, vmask_t, ident_t, c0_t, eps_t)
            for e in range(n_ex):
                emit_example(nc, pools, consts, xT_d.ap(), out_d.ap(), e)

    nc.compile()
    return nc


_cache = {}


def _get_program():
    if "nc" not in _cache:
        _cache["nc"] = build()
    return _cache["nc"]


def _run(x: np.ndarray, W: np.ndarray, **spmd_kwargs):
    x = np.asarray(x, np.float32)
    W = np.asarray(W, np.float32)
    nc = _get_program()
    xT = np.ascontiguousarray(x.transpose(0, 2, 1))  # [B, D, S]
    dmask, vmask = host_masks()
    in_maps = []
    for c in range(N_CORES):
        in_maps.append(
            {
                "xT": xT[c * E : (c + 1) * E],
                "W": W,
                "dmask": dmask,
                "vmask": vmask,
            }
        )
    res = run_bass_kernel_spmd(
        nc, in_maps, core_ids=list(range(N_CORES)), **spmd_kwargs
    )
    out = np.concatenate([res.results[c]["out"] for c in range(N_CORES)], axis=0)
    return out, res


def kernel(x: np.ndarray, W: np.ndarray) -> np.ndarray:
    return _run(x, W)[0]


# revision 9
# speedup vs baseline: 1.2327x; 1.2327x over previous
"""CapsNet dynamic-routing layer on 8 Trainium2 NeuronCores.

Math (per example, S=512 input capsules of dim D=256, 16 output capsules of
dim 32, O = 16*32 = 512):
  u_hat = x @ W                     # [S, O]
  b = 0; for 3 routing iters:
    c = softmax_n(b)                # over the 16-capsule axis
    s[n] = sum_s c[n,s] * u_hat[s, n*32:(n+1)*32]
    v = s / sqrt(|s|^2 + 1e-7)
    b[n,s] = v[n] . u_hat[s, n*32:(n+1)*32]
  out = v.flatten()

Sharding: pure data-parallel over the batch (256 examples -> 32 per core),
W replicated, no cross-core communication.

Layouts on chip (per example):
  u_hat  [S, O]  (S on partitions, 4 tiles) - rhs of the s-matmul
  u_hatT [O, S]  (O on partitions, 4 tiles) - rhs of the b-update matmul
Both are produced directly by the tensor engine from xT = x.T (host-side
transpose) since both matmuls contract over D:
  u_hat  tile m: sum_k  xT[k-tile][:, m-slice].T @ W[k-tile]
  u_hatT tile m: sum_k  W[k-tile][:, m-slice].T  @ xT[k-tile]
b is kept transposed as bT [S, 16] so softmax runs along the free dim.
"""

import sys

sys.path.insert(0, "/opt/trn_rl_repo")

import numpy as np

import concourse.bacc as bacc
import concourse.mybir as mybir
import concourse.tile as tile
from concourse import bass
from concourse.bass_utils import run_bass_kernel_spmd
from concourse.masks import make_identity

F32 = mybir.dt.float32
F32R = mybir.dt.float32r
AX = mybir.AxisListType
AF = mybir.ActivationFunctionType

B, S, D = 256, 512, 256
NC_, DC = 16, 32  # num_capsule, dim_capsule
O = NC_ * DC  # 512
N_CORES = 8
E = B // N_CORES  # 32 examples per core
ROUTINGS = 3
KT_D = D // 128  # 2 k-tiles over D
MT = 4  # 4 tiles over S and over O
RDT = F32R
NEW_SQUASH = True


def host_masks():
    # dmask[n, n'*32+d] = (n' == n): diagonal-block extraction mask
    dmask = np.zeros((NC_, O), np.float32)
    for n in range(NC_):
        dmask[n, n * DC : (n + 1) * DC] = 1.0
    # vmask[q, k*16+n'] = (n' == 4k + q//32): builds block-diag V from vT-replica
    vmask = np.zeros((128, 4 * NC_), np.float32)
    for q in range(128):
        for k in range(4):
            vmask[q, k * NC_ + 4 * k + q // 32] = 1.0
    return dmask, vmask


def emit_example(nc, pools, consts, xT_ap, out_ap, e):
    (xp, up, utp, sp, ctp, pcre, prt, ptr, pvt) = pools
    (W_t, dmask_t, vmask_t, ident_t, c0_t, eps_t) = consts

    # ---- load xT for this example: [D, S] as 2 partition tiles ----
    xt = []
    for k in range(KT_D):
        t = xp.tile([128, S], RDT, tag=f"xt{k}")
        nc.sync.dma_start(t[:], xT_ap[e, 128 * k : 128 * (k + 1), :])
        xt.append(t)

    # ---- u_hat [S, O] and u_hatT [O, S] ----
    uh, uhT = [], []
    for m in range(MT):
        pu = pcre.tile([128, O], F32, tag="pcre")
        for k in range(KT_D):
            nc.tensor.matmul(
                pu[:],
                xt[k][:, bass.ts(m, 128)],
                W_t[k][:],
                start=(k == 0),
                stop=(k == KT_D - 1),
            )
        t = up.tile([128, O], RDT, tag=f"uh{m}")
        nc.scalar.copy(t[:], pu[:])
        uh.append(t)
    for m in range(MT):
        pu = pcre.tile([128, S], F32, tag="pcre")
        for k in range(KT_D):
            nc.tensor.matmul(
                pu[:],
                W_t[k][:, bass.ts(m, 128)],
                xt[k][:],
                start=(k == 0),
                stop=(k == KT_D - 1),
            )
        t = utp.tile([128, S], RDT, tag=f"uht{m}")
        nc.scalar.copy(t[:], pu[:])
        uhT.append(t)

    # ---- routing ----
    cT = None  # [S, 16] as 4 tiles; iter 0 uses the constant 1/16 tile
    v = None
    for it in range(ROUTINGS):
        # s_full = cT.T @ u_hat : [16, O]
        ps = prt.tile([NC_, O], F32, tag="ps")
        for m in range(MT):
            lhs = c0_t if cT is None else cT[m]
            nc.tensor.matmul(
                ps[:], lhs[:], uh[m][:], start=(m == 0), stop=(m == MT - 1)
            )
        # diagonal-block extraction: s[n, d] = sum_n' masked[n, n'*32+d]
        masked = sp.tile([NC_, O], F32, tag="masked")
        nc.vector.tensor_mul(masked[:], ps[:], dmask_t[:])
        s = sp.tile([NC_, DC], F32, tag="s")
        nc.vector.tensor_reduce(
            s[:],
            masked[:].rearrange("p (n d) -> p d n", n=NC_),
            axis=AX.X,
            op=mybir.AluOpType.add,
        )
        # squash: v = s / sqrt(sum_d s^2 + 1e-7)
        sq = sp.tile([NC_, DC], F32, tag="sq")
        ss = sp.tile([NC_, 1], F32, tag="ss")
        inv = sp.tile([NC_, 1], F32, tag="inv")
        if NEW_SQUASH:
            nc.scalar.activation(sq[:], s[:], AF.Square, accum_out=ss[:])
            lnq = sp.tile([NC_, 1], F32, tag="lnq")
            nc.scalar.activation(lnq[:], ss[:], AF.Ln, bias=eps_t[:NC_, :])
            nc.scalar.activation(inv[:], lnq[:], AF.Exp, scale=-0.5)
        else:
            nc.scalar.activation(sq[:], s[:], AF.Square, accum_out=ss[:])
            nrm = sp.tile([NC_, 1], F32, tag="nrm")
            nc.scalar.activation(nrm[:], ss[:], AF.Sqrt, bias=eps_t[:NC_, :])
            nc.vector.reciprocal(inv[:], nrm[:])
        v = sp.tile([NC_, DC], F32, tag="v")
        nc.vector.tensor_scalar_mul(v[:], s[:], inv[:])

        if it == ROUTINGS - 1:
            break

        # ---- b update: bT' = (Vblk.T @ u_hatT) : [16, S] ----
        # Vblk [O, 16] block-diagonal, built from v via transpose + replicate + mask
        vtp = pvt.tile([DC, NC_], F32, tag="vtp")
        nc.tensor.transpose(vtp[:], v[:], ident_t[:NC_, :NC_])
        vrep = sp.tile([128, NC_], F32, tag="vrep")
        for r in range(4):
            nc.scalar.copy(vrep[32 * r : 32 * (r + 1), :], vtp[:])
        vblk = sp.tile([128, 4 * NC_], RDT, tag="vblk")
        for k in range(4):
            nc.vector.tensor_mul(
                vblk[:, bass.ts(k, NC_)], vmask_t[:, bass.ts(k, NC_)], vrep[:]
            )
        pb = prt.tile([NC_, S], F32, tag="ps")
        for k in range(MT):
            nc.tensor.matmul(
                pb[:],
                vblk[:, bass.ts(k, NC_)],
                uhT[k][:],
                start=(k == 0),
                stop=(k == MT - 1),
            )
        # softmax over n (free dim after transpose):
        # exp on the [16, S] layout, transpose 128-col slices to [S-tile, 16]
        expb = sp.tile([NC_, S], F32, tag="expb")
        nc.scalar.activation(expb[:], pb[:], AF.Exp)
        r4 = sp.tile([128, MT], F32, tag="r4")
        et_sb = []
        for m in range(MT):
            et = ptr.tile([128, NC_], F32, tag="et")
            nc.tensor.transpose(et[:], expb[:, bass.ts(m, 128)], ident_t[:NC_, :NC_])
            nc.vector.reduce_sum(r4[:, m : m + 1], et[:], axis=AX.X)
            esb = sp.tile([128, NC_], F32, tag=f"esb{m}")
            nc.scalar.copy(esb[:], et[:])
            et_sb.append(esb)
        inv4 = sp.tile([128, MT], F32, tag="inv4")
        nc.vector.reciprocal(inv4[:], r4[:])
        cT = []
        for m in range(MT):
            ct = ctp.tile([128, NC_], RDT, tag=f"ct{m}")
            nc.vector.tensor_scalar_mul(ct[:], et_sb[m][:], inv4[:, m : m + 1])
            cT.append(ct)

    # ---- output: v [16, 32] -> out[e] (row of 512) ----
    nc.sync.dma_start(out_ap[e].rearrange("(n d) -> n d", n=NC_), v[:])


def build(n_ex=E, num_devices=N_CORES, use_f32r=True, new_squash=True):
    global RDT, NEW_SQUASH
    RDT = F32R if use_f32r else F32
    NEW_SQUASH = new_squash
    nc = bacc.Bacc(
        "TRN2", target_bir_lowering=False, debug=False, num_devices=num_devices
    )
    xT_d = nc.dram_tensor("xT", [n_ex, D, S], RDT, kind="ExternalInput")
    W_d = nc.dram_tensor("W", [D, O], RDT, kind="ExternalInput")
    dmask_d = nc.dram_tensor("dmask", [NC_, O], F32, kind="ExternalInput")
    vmask_d = nc.dram_tensor("vmask", [128, 4 * NC_], F32, kind="ExternalInput")
    out_d = nc.dram_tensor("out", [n_ex, O], F32, kind="ExternalOutput")

    with tile.TileContext(nc) as tc:
        with (
            tc.tile_pool(name="consts", bufs=1) as cp,
            tc.tile_pool(name="xp", bufs=2) as xp,
            tc.tile_pool(name="up", bufs=2) as up,
            tc.tile_pool(name="utp", bufs=2) as utp,
            tc.tile_pool(name="sp", bufs=3) as sp,
            tc.tile_pool(name="ctp", bufs=2) as ctp,
            tc.tile_pool(name="pcre", bufs=2, space=bass.MemorySpace.PSUM) as pcre,
            tc.tile_pool(name="prt", bufs=2, space=bass.MemorySpace.PSUM) as prt,
            tc.tile_pool(name="ptr", bufs=2, space=bass.MemorySpace.PSUM) as ptr,
            tc.tile_pool(name="pvt", bufs=1, space=bass.MemorySpace.PSUM) as pvt,
        ):
            W_t = []
            for k in range(KT_D):
                t = cp.tile([128, O], RDT, tag=f"W{k}")
                nc.sync.dma_start(t[:], W_d.ap()[128 * k : 128 * (k + 1), :])
                W_t.append(t)
            dmask_t = cp.tile([NC_, O], F32, tag="dmask")
            nc.sync.dma_start(dmask_t[:], dmask_d.ap())
            vmask_t = cp.tile([128, 4 * NC_], F32, tag="vmask")
            nc.sync.dma_start(vmask_t[:], vmask_d.ap())
            ident_t = cp.tile([128, 128], F32, tag="ident")
            make_identity(nc, ident_t[:])
            c0_t = cp.tile([128, NC_], RDT, tag="c0")
            c0_f = cp.tile([128, NC_], F32, tag="c0f")
            nc.vector.memset(c0_f[:], 1.0 / NC_)
            nc.vector.tensor_copy(c0_t[:], c0_f[:])
            eps_t = cp.tile([128, 1], F32, tag="eps")
            nc.vector.memset(eps_t[:], 1e-7)

            pools = (xp, up, utp, sp, ctp, pcre, prt, ptr, pvt)
            consts = (W_t, dmask_t, vmask_t, ident_t, c0_t, eps_t)
            for e in range(n_ex):
                emit_example(nc, pools, consts, xT_d.ap(), out_d.ap(), e)

    nc.compile()
    return nc


_cache = {}


def _get_program():
    if "nc" not in _cache:
        _cache["nc"] = build()
    return _cache["nc"]


def _run(x: np.ndarray, W: np.ndarray, **spmd_kwargs):
    x = np.asarray(x, np.float32)
    W = np.asarray(W, np.float32)
    nc = _get_program()
    xT = np.ascontiguousarray(x.transpose(0, 2, 1))  # [B, D, S]
    dmask, vmask = host_masks()
    in_maps = []
    for c in range(N_CORES):
        in_maps.append(
            {
                "xT": xT[c * E : (c + 1) * E],
                "W": W,
                "dmask": dmask,
                "vmask": vmask,
            }
        )
    res = run_bass_kernel_spmd(
        nc, in_maps, core_ids=list(range(N_CORES)), **spmd_kwargs
    )
    out = np.concatenate([res.results[c]["out"] for c in range(N_CORES)], axis=0)
    return out, res


def kernel(x: np.ndarray, W: np.ndarray) -> np.ndarray:
    return _run(x, W)[0]


# revision 21
# speedup vs baseline: 3.2656x; 2.6491x over previous
"""CapsNet dynamic-routing layer on 8 Trainium2 NeuronCores.

Math (per example, S=512 input capsules of dim D=256, 16 output capsules of
dim 32, O = 16*32 = 512):
  u_hat = x @ W                     # [S, O]
  b = 0; for 3 routing iters:
    c = softmax_n(b)                # over the 16-capsule axis
    s[n] = sum_s c[n,s] * u_hat[s, n*32:(n+1)*32]
    v = s / sqrt(|s|^2 + 1e-7)
    b[n,s] = v[n] . u_hat[s, n*32:(n+1)*32]
  out = v.flatten()

Sharding: pure data-parallel over the batch (256 examples -> 32 per core),
W replicated, no cross-core communication.

Per-core structure: examples are processed in groups of 4 so that all the
thin [16, *] routing tensors pack into 32-partition strips of full
128-partition tiles (strip j holds example 4g+j; rows 16..31 of each strip
are dead). The four per-example routing matmuls of a K-tile go to four
different PE column groups (tile_position=(0, 32j)) and run concurrently.

Layouts (per example):
  u_hat  [S, O]  (S on partitions, 4 tiles) - rhs of the s-matmul
  u_hatT [O, S]  (O on partitions, 4 tiles) - rhs of the b-update matmul
Both come straight off the tensor engine from xT = x.T (host-transposed)
since both contract over D. b is kept transposed as bT [S, 16-per-ex] so
softmax runs along the free dim. Matmul operands use float32r (single-pass
fp32, ~1.6e-4 relative) unless use_f32r=False.
"""

import sys

sys.path.insert(0, "/opt/trn_rl_repo")

import numpy as np

import concourse.bacc as bacc
import concourse.mybir as mybir
import concourse.tile as tile
from concourse import bass
from concourse.bass_utils import run_bass_kernel_spmd
from concourse.masks import make_identity

F32 = mybir.dt.float32
F32R = mybir.dt.float32r
U32 = mybir.dt.uint32
AX = mybir.AxisListType
AF = mybir.ActivationFunctionType
OP = mybir.AluOpType

B, S, D = 256, 512, 256
NC_, DC = 16, 32  # num_capsule, dim_capsule
O = NC_ * DC  # 512
N_CORES = 8
E = B // N_CORES  # 32 examples per core
G = 4  # examples per group (one per PE column-group strip)
ROUTINGS = 3
KT_D = D // 128  # 2 k-tiles over D
MT = 4  # 4 tiles over S and over O
RDT = F32R  # matmul operand dtype (set by build())
QMAGIC = 0x5F3759DF  # rsqrt seed magic


def host_masks():
    # dmask4[32j+n, n'*32+d] = (n' == n) for n < 16, else 0 (strip pads dead)
    dmask4 = np.zeros((128, O), np.float32)
    for j in range(G):
        for n in range(NC_):
            dmask4[32 * j + n, n * DC : (n + 1) * DC] = 1.0
    # vmaskL[q, j*64 + k*16 + n'] = (n' == 4k + q//32), j-independent
    vmaskL = np.zeros((128, G * 4 * NC_), np.float32)
    for q in range(128):
        for j in range(G):
            for k in range(4):
                vmaskL[q, j * 64 + k * NC_ + 4 * k + q // 32] = 1.0
    return dmask4, vmaskL


def emit_group(nc, pools, consts, xT_ap, out_ap, g):
    (xp, up, utp, sp, ctp, pcre, pps, pet, pvt) = pools
    (W_t, dmask_t, vmask_t, ident_t, magic_t, c0_t) = consts

    # ---- load xT for 4 examples: [D, (e, S)] as 2 partition tiles ----
    xt = []
    for k in range(KT_D):
        t = xp.tile([128, G, S], RDT, tag=f"xt{k}")
        nc.sync.dma_start(
            t[:],
            xT_ap[G * g : G * (g + 1), 128 * k : 128 * (k + 1), :].rearrange(
                "e p s -> p e s"
            ),
        )
        xt.append(t)

    # ---- u_hat [S, O] and u_hatT [O, S] per example ----
    uh = [[None] * MT for _ in range(G)]
    uhT = [[None] * MT for _ in range(G)]
    for j in range(G):
        for m in range(MT):
            pu = pcre.tile([128, O], F32, tag="pcre")
            for k in range(KT_D):
                nc.tensor.matmul(
                    pu[:],
                    xt[k][:, j, bass.ts(m, 128)],
                    W_t[k][:],
                    start=(k == 0),
                    stop=(k == KT_D - 1),
                )
            t = up.tile([128, O], RDT, tag=f"uh{j}{m}")
            nc.scalar.copy(t[:], pu[:])
            uh[j][m] = t
        for m in range(MT):
            pu = pcre.tile([128, S], F32, tag="pcre")
            for k in range(KT_D):
                nc.tensor.matmul(
                    pu[:],
                    W_t[k][:, bass.ts(m, 128)],
                    xt[k][:, j, :],
                    start=(k == 0),
                    stop=(k == KT_D - 1),
                )
            t = utp.tile([128, S], RDT, tag=f"uht{j}{m}")
            nc.scalar.copy(t[:], pu[:])
            uhT[j][m] = t

    # ---- routing (4 examples packed in 32-partition strips) ----
    cT = None  # [S-chunk m] -> [128, (j,16)] compact; iter 0 uses 1/16 const
    v = None
    for it in range(ROUTINGS):
        # s_full_j = cT_j.T @ u_hat_j : own [16, O] psum per example, then a
        # fused mask+gather packs the strips into one [128, O] sbuf tile
        # (fp32-family matmuls cannot write PSUM at partition offset != 0)
        masked = sp.tile([128, O], F32, tag="masked")
        nc.vector.memset(masked[:], 0.0)
        for j in range(G):
            ps = pps.tile([NC_, O], F32, tag="ps")
            for m in range(MT):
                lhs = (
                    c0_t[:]
                    if cT is None
                    else cT[m][:, NC_ * j : NC_ * (j + 1)]
                )
                nc.tensor.matmul(
                    ps[:],
                    lhs,
                    uh[j][m][:],
                    start=(m == 0),
                    stop=(m == MT - 1),
                )
            nc.vector.tensor_mul(
                masked[32 * j : 32 * j + NC_, :], ps[:], dmask_t[: NC_, :]
            )
        s = sp.tile([128, DC], F32, tag="s")
        nc.vector.tensor_reduce(
            s[:],
            masked[:].rearrange("p (n d) -> p d n", n=NC_),
            axis=AX.X,
            op=OP.add,
        )
        # squash: v = s * rsqrt(|s|^2 + 1e-7); rsqrt = quake seed + 3 Newton
        sq = sp.tile([128, DC], F32, tag="sq")
        ss = sp.tile([128, 1], F32, tag="ss")
        nc.scalar.activation(sq[:], s[:], AF.Square, accum_out=ss[:])
        q = sp.tile([128, 1], F32, tag="q")
        nc.vector.tensor_scalar_add(q[:], ss[:], 1e-7)
        sh = sp.tile([128, 1], U32, tag="sh")
        nc.vector.tensor_scalar(
            sh[:], q[:].bitcast(U32), 1, None, op0=OP.logical_shift_right
        )
        y = sp.tile([128, 1], F32, tag="y")
        nc.vector.tensor_tensor(
            y[:].bitcast(U32), magic_t[:], sh[:], op=OP.subtract
        )
        for _ in range(3):
            t2 = sp.tile([128, 1], F32, tag="t2")
            nc.vector.tensor_tensor(t2[:], y[:], y[:], op=OP.mult)
            nc.vector.tensor_tensor(t2[:], t2[:], q[:], op=OP.mult)
            nc.vector.tensor_scalar(
                t2[:], t2[:], -0.5, 1.5, op0=OP.mult, op1=OP.add
            )
            nc.vector.tensor_tensor(y[:], y[:], t2[:], op=OP.mult)
        v = sp.tile([128, DC], F32, tag="v")
        nc.vector.tensor_scalar_mul(v[:], s[:], y[:])

        if it == ROUTINGS - 1:
            break

        # ---- b update: bT'[strip j] = Vblk_j.T @ u_hatT_j ----
        vtp = pvt.tile([DC, 128], F32, tag="vtp")
        nc.tensor.transpose(vtp[:], v[:], ident_t[:])
        vv = sp.tile([128, G * 4], F32, tag="vv")
        vtp_jx = vtp[:].rearrange("p (j x) -> p j x", j=G)
        for r in range(4):  # strip row n_lo = r: VV[32r+d,(j,k)] = vtp[d,32j+4k+r]
            nc.vector.tensor_copy(
                vv[32 * r : 32 * (r + 1), :].rearrange("p (j k) -> p j k", j=G),
                vtp_jx[:, :, r : NC_ : 4],
            )
        vblk = sp.tile([128, G * 4 * NC_], RDT, tag="vblk")
        nc.vector.tensor_mul(
            vblk[:].rearrange("p (j k n) -> p j k n", j=G, k=4),
            vmask_t[:].rearrange("p (j k n) -> p j k n", j=G, k=4),
            vv[:]
            .rearrange("p (j k one) -> p j k one", j=G, one=1)
            .to_broadcast([128, G, 4, NC_]),
        )
        # bT'_j = Vblk_j.T @ u_hatT_j in its own [16, S] psum; the per-strip
        # exp packs results into one [128, S] sbuf tile
        expb = sp.tile([128, S], F32, tag="expb")
        nc.vector.memset(expb[:], 0.0)
        for j in range(G):
            pb = pps.tile([NC_, S], F32, tag="ps")
            for k in range(MT):
                nc.tensor.matmul(
                    pb[:],
                    vblk[:, 64 * j + NC_ * k : 64 * j + NC_ * (k + 1)],
                    uhT[j][k][:],
                    start=(k == 0),
                    stop=(k == MT - 1),
                )
            nc.scalar.activation(
                expb[32 * j : 32 * j + NC_, :], pb[:], AF.Exp
            )
        et = pet.tile([128, MT, 128], F32, tag="et")
        r_all = sp.tile([128, MT * G], F32, tag="r_all")
        for m in range(MT):
            nc.tensor.transpose(et[:, m, :], expb[:, bass.ts(m, 128)], ident_t[:])
            nc.vector.tensor_reduce(
                r_all[:, G * m : G * (m + 1)],
                et[:, m, :].rearrange("p (j n) -> p j n", j=G)[:, :, :NC_],
                axis=AX.X,
                op=OP.add,
            )
        rinv = sp.tile([128, MT * G], F32, tag="rinv")
        nc.vector.reciprocal(rinv[:], r_all[:])
        cT = []
        for m in range(MT):
            ct = ctp.tile([128, G * NC_], RDT, tag=f"ct{m}")
            nc.vector.tensor_mul(
                ct[:].rearrange("p (j n) -> p j n", j=G),
                et[:, m, :].rearrange("p (j n) -> p j n", j=G)[:, :, :NC_],
                rinv[:, G * m : G * (m + 1)]
                .rearrange("p (j one) -> p j one", one=1)
                .to_broadcast([128, G, NC_]),
            )
            cT.append(ct)

    # ---- output: strip j -> row 4g+j ----
    for j in range(G):
        nc.sync.dma_start(
            out_ap[G * g + j].rearrange("(n d) -> n d", n=NC_),
            v[32 * j : 32 * j + NC_, :],
        )


def build(n_ex=E, num_devices=N_CORES, use_f32r=True):
    global RDT
    RDT = F32R if use_f32r else F32
    assert n_ex % G == 0
    nc = bacc.Bacc(
        "TRN2", target_bir_lowering=False, debug=False, num_devices=num_devices
    )
    xT_d = nc.dram_tensor("xT", [n_ex, D, S], RDT, kind="ExternalInput")
    W_d = nc.dram_tensor("W", [D, O], RDT, kind="ExternalInput")
    dmask_d = nc.dram_tensor("dmask", [128, O], F32, kind="ExternalInput")
    vmask_d = nc.dram_tensor("vmask", [128, G * 4 * NC_], F32, kind="ExternalInput")
    out_d = nc.dram_tensor("out", [n_ex, O], F32, kind="ExternalOutput")

    with tile.TileContext(nc) as tc:
        with (
            tc.tile_pool(name="consts", bufs=1) as cp,
            tc.tile_pool(name="xp", bufs=2) as xp,
            tc.tile_pool(name="up", bufs=2) as up,
            tc.tile_pool(name="utp", bufs=2) as utp,
            tc.tile_pool(name="sp", bufs=3) as sp,
            tc.tile_pool(name="ctp", bufs=2) as ctp,
            tc.tile_pool(name="pcre", bufs=2, space=bass.MemorySpace.PSUM) as pcre,
            tc.tile_pool(name="pps", bufs=3, space=bass.MemorySpace.PSUM) as pps,
            tc.tile_pool(name="pet", bufs=2, space=bass.MemorySpace.PSUM) as pet,
            tc.tile_pool(name="pvt", bufs=1, space=bass.MemorySpace.PSUM) as pvt,
        ):
            W_t = []
            for k in range(KT_D):
                t = cp.tile([128, O], RDT, tag=f"W{k}")
                nc.sync.dma_start(t[:], W_d.ap()[128 * k : 128 * (k + 1), :])
                W_t.append(t)
            dmask_t = cp.tile([128, O], F32, tag="dmask")
            nc.sync.dma_start(dmask_t[:], dmask_d.ap())
            vmask_t = cp.tile([128, G * 4 * NC_], F32, tag="vmask")
            nc.sync.dma_start(vmask_t[:], vmask_d.ap())
            ident_t = cp.tile([128, 128], F32, tag="ident")
            make_identity(nc, ident_t[:])
            magic_t = cp.tile([128, 1], U32, tag="magic")
            nc.vector.memset(magic_t[:], QMAGIC)
            c0_t = cp.tile([128, NC_], RDT, tag="c0")
            c0_f = cp.tile([128, NC_], F32, tag="c0f")
            nc.vector.memset(c0_f[:], 1.0 / NC_)
            nc.vector.tensor_copy(c0_t[:], c0_f[:])

            pools = (xp, up, utp, sp, ctp, pcre, pps, pet, pvt)
            consts = (W_t, dmask_t, vmask_t, ident_t, magic_t, c0_t)
            for g in range(n_ex // G):
                emit_group(nc, pools, consts, xT_d.ap(), out_d.ap(), g)

    nc.compile()
    return nc


_cache = {}


def _get_program():
    if "nc" not in _cache:
        _cache["nc"] = build()
    return _cache["nc"]


def _run(x: np.ndarray, W: np.ndarray, **spmd_kwargs):
    x = np.asarray(x, np.float32)
    W = np.asarray(W, np.float32)
    nc = _get_program()
    xT = np.ascontiguousarray(x.transpose(0, 2, 1))  # [B, D, S]
    dmask, vmask = host_masks()
    in_maps = []
    for c in range(N_CORES):
        in_maps.append(
            {
                "xT": xT[c * E : (c + 1) * E],
                "W": W,
                "dmask": dmask,
                "vmask": vmask,
            }
        )
    res = run_bass_kernel_spmd(
        nc, in_maps, core_ids=list(range(N_CORES)), **spmd_kwargs
    )
    out = np.concatenate([res.results[c]["out"] for c in range(N_CORES)], axis=0)
    return out, res


def kernel(x: np.ndarray, W: np.ndarray) -> np.ndarray:
    return _run(x, W)[0]
